# revision 17
# baseline (speedup 1.0000x reference)
"""Trainium2 Bass kernel for nn_CostFn_18562848653837.

reference(x, cond, time) only reads x[b, j, 6+k] for j in [0,26), k in [0,6)
(~2.6 MB of the 436 MB input; cond/time are unused) and computes, per point,
the reflected mass 1 / (u^T J M^{-1} J^T u) with u = e_x, which reduces via
Sherman-Morrison (M = 2I + 0.5 c c^T) to pure functions of sin^2(cq) and
sin(2*cq), cq = cumsum(q):

    Q1 = sum_k L_k^2 sin^2(cq_k)
    Q3 = sum_k sin^2(cq_k)
    P2 = sum_k L_k sin(2 cq_k)
    TC = 2.5 - 0.25*Q3
    cost = TC / (0.5*Q1*TC - P2^2/32)

Both sin^2(th) and sin(2 th) are invariant under th -> th - k*pi, so the
host ships m = cq/pi - rne(cq/pi) in [-0.5, 0.5] as bf16 (phase error
~2^-10 pi; checked: total rel err ~1e-5 vs the 2e-2 gate), halving DMA
bytes. The device does all the nonlinear math: both Sins on ACT (the
radians conversion rides the ACT input scale; |pi*m| <= pi/2 and
|2pi(1-2^-23)m| < pi keep the table domain satisfied), squares + the three
weighted 6-plane accumulations, TB, and the final
G -> D -> reciprocal -> cost chain with the column sum fused into the last
scalar_tensor_tensor via accum_out.

Engine split: ACT 4 Sin chunks (~1.6us); Pool SQ_k + NTC (= -TC) chain +
P2 chain + TB (19 ops, ~1.65us); DVE Q1 chain + G/D/R/FIN tail (~1.7us).
TC is accumulated negated (NTC = -2.5 + sum 0.25*SQ_k) because the ALU
set has no reversed subtract; downstream stt scalars absorb the sign.

Sharding: pure data parallel over batch - core i gets batches
[512*i, 512*(i+1)), i.e. 13312 points as (128, 104) per q-component.
Each core emits a (128,1) f32 partial; host adds the 8*128 values.
"""

import numpy as np

_P, _W, _K = 128, 104, 6
_NCORES = 8
_B, _H, _T = 4096, 1024, 26
_BPC = _B // _NCORES  # batches per core
_NCHUNK = 2           # DMA chunks, 3 planes each
_CW = 3 * _W          # chunk width (312)

_CACHE = {}


def _get_d_op():
    """Register (once) a fused custom DVE op: D = in0 - s0*in1^2.
    Folds the P2^2/32 term and the subtract into one Vector pass."""
    if "d_op" in _CACHE:
        return _CACHE["d_op"]
    import concourse.dve_ops as dve_ops
    from concourse.dve_ops import DveOp, OPS, CUSTOM_DVE_SPECS, _SUB_OPCODE_FOR_NAME
    from concourse.dve_spec import Spec, Src0, Src1, C0, sq, lower
    from concourse.dve_uop import DveOpSpec

    name = "SUB_SCALED_SQ_ANT"
    if name not in _SUB_OPCODE_FOR_NAME:
        spec = Spec(
            body=Src0 - sq(Src1) * C0,
            reference=lambda in0, in1, s0, s1, imm2: in0.astype(np.float32)
            - (in1.astype(np.float32) * in1) * s0,
        )
        row = max(_SUB_OPCODE_FOR_NAME.values()) + 1
        assert row < 0x20
        shas = {}
        for ver in ("v3", "v4"):
            shas[ver] = DveOpSpec(
                name=name, opcode=row, uops=lower(spec, ver=ver), rd1_en=True
            ).sha(ver)
        op = DveOp(name, spec, subdim=False, uops_sha=shas)
        OPS.append(op)
        CUSTOM_DVE_SPECS[name] = spec
        _SUB_OPCODE_FOR_NAME[name] = row
        _CACHE["d_op"] = op
    else:
        _CACHE["d_op"] = next(o for o in dve_ops.OPS if o.name == name)
    return _CACHE["d_op"]


def _get_nc():
    if "nc" in _CACHE:
        return _CACHE["nc"]

    import concourse.tile as tile
    import concourse.mybir as mybir
    from concourse import bacc

    PI32 = float(np.float32(np.pi))
    # One-ulp-shaded 2*pi: |m| <= 0.5 exactly, so the Sin input
    # |SCALE2*m| <= pi*(1-2^-23) stays strictly inside the table domain.
    SCALE2 = float(np.float32(2.0 * np.pi * (1.0 - 2.0**-23)))
    L = [float(np.float32(v)) for v in np.arange(1, 7) * 0.1 + 0.3]

    f32 = mybir.dt.float32
    bf16 = mybir.dt.bfloat16
    OP = mybir.AluOpType
    ACT = mybir.ActivationFunctionType

    nc = bacc.Bacc(
        "TRN2", target_bir_lowering=False, debug=False, num_devices=_NCORES,
        disable_frame_to_traceback=True,
    )
    q_dram = nc.dram_tensor("q", [_NCHUNK, _P, _CW], bf16, kind="ExternalInput")
    out_dram = nc.dram_tensor("out", [_P, 1], f32, kind="ExternalOutput")

    d_op = _get_d_op()

    with (
        tile.TileContext(nc) as tc,
        tc.tile_pool(name="pool", bufs=1) as pool,
    ):
        # Dep-free dummy Sin on the const-1.0 AP: hoists the ~1.3us Sin
        # table-set load to t~300 (right after the Bacc init barrier),
        # hidden behind the input DMAs.
        one_ap = nc.const_aps.aps[(f32, 1.0)]
        WARM = pool.tile([_P, 1], f32)
        nc.scalar.activation(WARM[:], one_ap[:_P], ACT.Sin)

        # Two input chunks (planes 0-2 / 3-5) on the two DMA-capable
        # sequencers so both issue in parallel.
        Qc = []
        for c in range(_NCHUNK):
            qc = pool.tile([_P, _CW], bf16, tag=f"q{c}")
            eng = nc.sync if c == 0 else nc.gpsimd
            eng.dma_start(qc[:], q_dram[c])
            Qc.append(qc)

        # ACT: SM chunks first (their downstream SQ->{Q3,TC,Q1,G} chain is
        # the long one), SF chunks after (their P2 tail is shorter).
        SM = pool.tile([_P, _K * _W], bf16)
        SF = pool.tile([_P, _K * _W], bf16)
        for c in range(_NCHUNK):
            sl = slice(c * _CW, (c + 1) * _CW)
            nc.scalar.activation(SM[:, sl], Qc[c][:], ACT.Sin, scale=PI32)
        for c in range(_NCHUNK):
            sl = slice(c * _CW, (c + 1) * _CW)
            nc.scalar.activation(SF[:, sl], Qc[c][:], ACT.Sin, scale=SCALE2)

        # Pool (tensor_scalar/tensor_tensor only - stt is DVE-only on HW):
        # squares, Q3 accumulation, TC; DVE: Q1 stt chain.
        SQ = pool.tile([_P, _K * _W], bf16)
        Q3 = pool.tile([_P, _W], bf16)
        Q1 = pool.tile([_P, _W], bf16)
        q3_tail = []  # Pool ops that may yield to the P2a block
        for k in range(_K):
            sl = slice(k * _W, (k + 1) * _W)
            sq_i = nc.gpsimd.tensor_mul(SQ[:, sl], SM[:, sl], SM[:, sl])
            if k == 1:
                q3_i = nc.gpsimd.tensor_add(Q3[:], SQ[:, 0:_W], SQ[:, _W : 2 * _W])
                q3_tail.append(q3_i)
            elif k > 1:
                q3_i = nc.gpsimd.tensor_add(Q3[:], Q3[:], SQ[:, sl])
                q3_tail.append(q3_i)
            del sq_i
            if k == 0:
                nc.vector.tensor_scalar_mul(Q1[:], SQ[:, sl], L[0] * L[0])
            else:
                nc.vector.scalar_tensor_tensor(
                    Q1[:], SQ[:, sl], L[k] * L[k], Q1[:], OP.mult, OP.add
                )
        TC = pool.tile([_P, _W], f32)
        nc.gpsimd.tensor_scalar(TC[:], Q3[:], -0.25, 2.5, OP.mult, OP.add)

        # G = 0.5*Q1*TC as two Pool-legal ops (fills Pool slack; keeps DVE
        # free for the P2 tail chain).
        H = pool.tile([_P, _W], f32)
        nc.gpsimd.tensor_scalar_mul(H[:], Q1[:], 0.5)
        G = pool.tile([_P, _W], f32)
        nc.gpsimd.tensor_mul(G[:], H[:], TC[:])

        # P2: planes 0-2 accumulate on Pool right off SF012; planes 3-5
        # continue as a DVE stt chain straight into the D/R/FIN tail.
        WS = pool.tile([_P, 3 * _W], bf16)
        p2a_block = []
        for k in range(3):
            sl = slice(k * _W, (k + 1) * _W)
            p2a_block.append(nc.gpsimd.tensor_scalar_mul(WS[:, sl], SF[:, sl], L[k]))
        PA = pool.tile([_P, _W], bf16)
        p2a_block.append(nc.gpsimd.tensor_add(PA[:], WS[:, 0:_W], WS[:, _W : 2 * _W]))
        P2a = pool.tile([_P, _W], bf16)
        p2a_last = nc.gpsimd.tensor_add(P2a[:], PA[:], WS[:, 2 * _W : 3 * _W])
        p2a_block.append(p2a_last)
        # order-only edges: once SF012 lands, the P2a block takes priority
        # on Pool over the remaining Q3/SQ tail (whose consumers G/TC are
        # only needed ~700ns later)
        for late in q3_tail[-3:]:
            for early in p2a_block[-2:]:
                tile.add_dep_helper(
                    late.ins, early.ins, sync=False,
                    reason="P2a block before Q3 tail",
                )
        P2 = pool.tile([_P, _W], bf16)
        nc.vector.scalar_tensor_tensor(
            P2[:], SF[:, 3 * _W : 4 * _W], L[3], P2a[:], OP.mult, OP.add
        )
        nc.vector.scalar_tensor_tensor(
            P2[:], SF[:, 4 * _W : 5 * _W], L[4], P2[:], OP.mult, OP.add
        )
        nc.vector.scalar_tensor_tensor(
            P2[:], SF[:, 5 * _W : 6 * _W], L[5], P2[:], OP.mult, OP.add
        )

        # DVE tail: D = G - P2^2/32 (fused custom op); R ~= 1/D (1-pass
        # approx, ~51 ULP); cost = R*TC with the column sum fused via
        # accum_out.
        D = pool.tile([_P, _W], f32)
        nc.vector._custom_dve(d_op, out=D[:], in0=G[:], in1=P2[:], s0=1.0 / 32.0)
        R = pool.tile([_P, _W], f32)
        nc.vector.reciprocal_approx_fast(out=R[:], in_=D[:])
        COST = pool.tile([_P, _W], f32)
        CS = pool.tile([_P, 1], f32)
        nc.vector.scalar_tensor_tensor(
            COST[:], R[:], 1.0, TC[:], OP.mult, OP.mult, accum_out=CS[:]
        )
        nc.sync.dma_start(out_dram[:], CS[:])

    nc.compile()
    _CACHE["nc"] = nc
    return nc


def _shard(x):
    """Host prep: slice, cumsum over joints, /pi, RNE range-reduce to
    [-0.5,0.5], bf16, and lay out per core as [2 chunks, 128, 3*104]
    (chunk c = planes 3c..3c+2, k-major within the chunk)."""
    import ml_dtypes

    q = np.asarray(x[:, :_T, 6 : 6 + _K], dtype=np.float32)
    g = np.cumsum(q, axis=-1, dtype=np.float32) * np.float32(1.0 / np.pi)
    m = (g - np.rint(g)).astype(np.float32)
    # (B, T, K) -> (cores, K, P, W) point-major layout per plane
    planes = (
        m.reshape(_NCORES, _BPC * _T, _K)
        .transpose(0, 2, 1)
        .reshape(_NCORES, _K, _P, _W)
    )
    # group planes into chunks of 3: (cores, 2, 128, 312)
    chunks = (
        planes.reshape(_NCORES, _NCHUNK, 3, _P, _W)
        .transpose(0, 1, 3, 2, 4)
        .reshape(_NCORES, _NCHUNK, _P, _CW)
    )
    return np.ascontiguousarray(chunks.astype(ml_dtypes.bfloat16))


def _get_runner():
    """Build the jitted 8-core shard_map executable once (mirrors
    bass2jax.run_bass_via_pjrt's multi-core path) so repeat kernel() calls
    skip retracing/recompiling."""
    if "run" in _CACHE:
        return _CACHE["run"]
    import jax
    from jax.sharding import Mesh, PartitionSpec
    from jax.experimental.shard_map import shard_map
    from concourse import bass2jax

    nc = _get_nc()
    bass2jax.install_neuronx_cc_hook()
    assert nc.dbg_addr is None
    pid_name = nc.partition_id_tensor.name if nc.partition_id_tensor else None
    in_names = ("q", "out") + ((pid_name,) if pid_name else ())

    out_aval = jax.core.ShapedArray((_P, 1), np.float32)

    def _body(q, out_zero):
        operands = [q, out_zero]
        if pid_name is not None:
            operands.append(bass2jax.partition_id_tensor())
        (out,) = bass2jax._bass_exec_p.bind(
            *operands,
            out_avals=(out_aval,),
            in_names=in_names,
            out_names=("out",),
            lowering_input_output_aliases=(),
            sim_require_finite=True,
            sim_require_nnan=True,
            nc=nc,
        )
        return (out,)

    devices = jax.devices()[:_NCORES]
    mesh = Mesh(np.asarray(devices), ("core",))
    sharded = jax.jit(
        shard_map(
            _body,
            mesh=mesh,
            in_specs=(PartitionSpec("core"),) * 2,
            out_specs=(PartitionSpec("core"),),
            check_rep=False,
        ),
        donate_argnums=(1,),
        keep_unused=True,
    )

    def run(planes):
        concat_q = planes.reshape(_NCORES * _NCHUNK, _P, _CW)
        zeros = np.zeros((_NCORES * _P, 1), np.float32)
        (out,) = sharded(concat_q, zeros)
        return np.asarray(out)  # (8*128, 1)

    _CACHE["run"] = run
    return run


def _run_library(planes):
    from concourse.bass_utils import run_bass_kernel_spmd

    res = run_bass_kernel_spmd(
        _get_nc(),
        [{"q": planes[i]} for i in range(_NCORES)],
        list(range(_NCORES)),
    )
    return np.stack([r["out"][:, 0] for r in res.results]).astype(np.float32)


def _run_subprocess(planes):
    """Last resort: the accelerator occasionally reports
    NRT_EXEC_UNIT_UNRECOVERABLE; a fresh process reliably recovers it."""
    import os
    import subprocess
    import sys
    import tempfile

    d = tempfile.mkdtemp()
    inp = os.path.join(d, "planes.npy")
    out = os.path.join(d, "out.npy")
    np.save(inp, planes)
    here = os.path.dirname(os.path.abspath(__file__))
    script = (
        "import sys, numpy as np\n"
        f"sys.path.insert(0, {here!r})\n"
        "import kernel as K\n"
        f"planes = np.load({inp!r})\n"
        "out = K._get_runner()(planes)\n"
        f"np.save({out!r}, out)\n"
    )
    err = None
    for _ in range(2):
        try:
            subprocess.run(
                [sys.executable, "-c", script], check=True, timeout=900,
                stdout=subprocess.DEVNULL, stderr=subprocess.DEVNULL,
            )
            return np.load(out).astype(np.float32)
        except Exception as e:  # retry once; device usually recovers
            err = e
    raise err


def kernel(x, cond, time):
    x = np.asarray(x)
    planes = _shard(x)
    try:
        partials = _get_runner()(planes).astype(np.float32)
    except Exception:
        try:
            # library SPMD runner (covers fast-path/jax API drift)
            partials = _run_library(planes)
        except Exception:
            # fresh process recovers a wedged accelerator
            partials = _run_subprocess(planes)
    return np.float32(partials.sum(dtype=np.float32))


# revision 18
# speedup vs baseline: 1.0025x; 1.0025x over previous
"""Trainium2 Bass kernel for nn_CostFn_18562848653837.

reference(x, cond, time) only reads x[b, j, 6+k] for j in [0,26), k in [0,6)
(~2.6 MB of the 436 MB input; cond/time are unused) and computes, per point,
the reflected mass 1 / (u^T J M^{-1} J^T u) with u = e_x, which reduces via
Sherman-Morrison (M = 2I + 0.5 c c^T) to pure functions of sin^2(cq) and
sin(2*cq), cq = cumsum(q):

    Q1 = sum_k L_k^2 sin^2(cq_k)
    Q3 = sum_k sin^2(cq_k)
    P2 = sum_k L_k sin(2 cq_k)
    TC = 2.5 - 0.25*Q3
    cost = TC / (0.5*Q1*TC - P2^2/32)

Both sin^2(th) and sin(2 th) are invariant under th -> th - k*pi, so the
host ships m = cq/pi - rne(cq/pi) in [-0.5, 0.5] as bf16 (phase error
~2^-10 pi; checked: total rel err ~1e-5 vs the 2e-2 gate), halving DMA
bytes. The device does all the nonlinear math: both Sins on ACT (the
radians conversion rides the ACT input scale; |pi*m| <= pi/2 and
|2pi(1-2^-23)m| < pi keep the table domain satisfied), squares + the three
weighted 6-plane accumulations, TB, and the final
G -> D -> reciprocal -> cost chain with the column sum fused into the last
scalar_tensor_tensor via accum_out.

Engine split: ACT 4 Sin chunks (~1.6us); Pool SQ_k + NTC (= -TC) chain +
P2 chain + TB (19 ops, ~1.65us); DVE Q1 chain + G/D/R/FIN tail (~1.7us).
TC is accumulated negated (NTC = -2.5 + sum 0.25*SQ_k) because the ALU
set has no reversed subtract; downstream stt scalars absorb the sign.

Sharding: pure data parallel over batch - core i gets batches
[512*i, 512*(i+1)), i.e. 13312 points as (128, 104) per q-component.
Each core emits a (128,1) f32 partial; host adds the 8*128 values.
"""

import numpy as np

_P, _W, _K = 128, 104, 6
_NCORES = 8
_B, _H, _T = 4096, 1024, 26
_BPC = _B // _NCORES  # batches per core
_NCHUNK = 2           # DMA chunks, 3 planes each
_CW = 3 * _W          # chunk width (312)

_CACHE = {}


def _get_d_op():
    """Register (once) a fused custom DVE op: D = in0 - s0*in1^2.
    Folds the P2^2/32 term and the subtract into one Vector pass."""
    if "d_op" in _CACHE:
        return _CACHE["d_op"]
    import concourse.dve_ops as dve_ops
    from concourse.dve_ops import DveOp, OPS, CUSTOM_DVE_SPECS, _SUB_OPCODE_FOR_NAME
    from concourse.dve_spec import Spec, Src0, Src1, C0, sq, lower
    from concourse.dve_uop import DveOpSpec

    name = "SUB_SCALED_SQ_ANT"
    if name not in _SUB_OPCODE_FOR_NAME:
        spec = Spec(
            body=Src0 - sq(Src1) * C0,
            reference=lambda in0, in1, s0, s1, imm2: in0.astype(np.float32)
            - (in1.astype(np.float32) * in1) * s0,
        )
        row = max(_SUB_OPCODE_FOR_NAME.values()) + 1
        assert row < 0x20
        shas = {}
        for ver in ("v3", "v4"):
            shas[ver] = DveOpSpec(
                name=name, opcode=row, uops=lower(spec, ver=ver), rd1_en=True
            ).sha(ver)
        op = DveOp(name, spec, subdim=False, uops_sha=shas)
        OPS.append(op)
        CUSTOM_DVE_SPECS[name] = spec
        _SUB_OPCODE_FOR_NAME[name] = row
        _CACHE["d_op"] = op
    else:
        _CACHE["d_op"] = next(o for o in dve_ops.OPS if o.name == name)
    return _CACHE["d_op"]


def _get_nc():
    if "nc" in _CACHE:
        return _CACHE["nc"]

    import concourse.tile as tile
    import concourse.mybir as mybir
    from concourse import bacc

    PI32 = float(np.float32(np.pi))
    # One-ulp-shaded 2*pi: |m| <= 0.5 exactly, so the Sin input
    # |SCALE2*m| <= pi*(1-2^-23) stays strictly inside the table domain.
    SCALE2 = float(np.float32(2.0 * np.pi * (1.0 - 2.0**-23)))
    L = [float(np.float32(v)) for v in np.arange(1, 7) * 0.1 + 0.3]

    f32 = mybir.dt.float32
    bf16 = mybir.dt.bfloat16
    OP = mybir.AluOpType
    ACT = mybir.ActivationFunctionType

    nc = bacc.Bacc(
        "TRN2", target_bir_lowering=False, debug=False, num_devices=_NCORES,
        disable_frame_to_traceback=True,
    )
    q_dram = nc.dram_tensor("q", [_NCHUNK, _P, _CW], bf16, kind="ExternalInput")
    out_dram = nc.dram_tensor("out", [_P, 1], f32, kind="ExternalOutput")

    d_op = _get_d_op()

    with (
        tile.TileContext(nc) as tc,
        tc.tile_pool(name="pool", bufs=1) as pool,
    ):
        # Dep-free dummy Sin on the const-1.0 AP: hoists the ~1.3us Sin
        # table-set load to t~300 (right after the Bacc init barrier),
        # hidden behind the input DMAs.
        one_ap = nc.const_aps.aps[(f32, 1.0)]
        WARM = pool.tile([_P, 1], f32)
        nc.scalar.activation(WARM[:], one_ap[:_P], ACT.Sin)

        # Two input chunks (planes 0-2 / 3-5) on the two DMA-capable
        # sequencers so both issue in parallel.
        Qc = []
        for c in range(_NCHUNK):
            qc = pool.tile([_P, _CW], bf16, tag=f"q{c}")
            eng = nc.sync if c == 0 else nc.gpsimd
            eng.dma_start(qc[:], q_dram[c])
            Qc.append(qc)

        # ACT: SM chunks first (their downstream SQ->{Q3,TC,Q1,G} chain is
        # the long one), SF chunks after (their P2 tail is shorter).
        SM = pool.tile([_P, _K * _W], bf16)
        SF = pool.tile([_P, _K * _W], bf16)
        for c in range(_NCHUNK):
            sl = slice(c * _CW, (c + 1) * _CW)
            nc.scalar.activation(SM[:, sl], Qc[c][:], ACT.Sin, scale=PI32)
        for c in range(_NCHUNK):
            sl = slice(c * _CW, (c + 1) * _CW)
            nc.scalar.activation(SF[:, sl], Qc[c][:], ACT.Sin, scale=SCALE2)

        # Pool (tensor_scalar/tensor_tensor only - stt is DVE-only on HW):
        # squares, Q3 accumulation, TC; DVE: Q1 stt chain.
        SQ = pool.tile([_P, _K * _W], bf16)
        Q3 = pool.tile([_P, _W], bf16)
        Q1 = pool.tile([_P, _W], bf16)
        q3_tail = []  # Pool ops that may yield to the P2a block
        for k in range(_K):
            sl = slice(k * _W, (k + 1) * _W)
            sq_i = nc.gpsimd.tensor_mul(SQ[:, sl], SM[:, sl], SM[:, sl])
            if k == 1:
                q3_i = nc.gpsimd.tensor_add(Q3[:], SQ[:, 0:_W], SQ[:, _W : 2 * _W])
                q3_tail.append(q3_i)
            elif k > 1:
                q3_i = nc.gpsimd.tensor_add(Q3[:], Q3[:], SQ[:, sl])
                q3_tail.append(q3_i)
            del sq_i
            if k == 0:
                nc.vector.tensor_scalar_mul(Q1[:], SQ[:, sl], L[0] * L[0])
            else:
                nc.vector.scalar_tensor_tensor(
                    Q1[:], SQ[:, sl], L[k] * L[k], Q1[:], OP.mult, OP.add
                )
        TC = pool.tile([_P, _W], f32)
        nc.gpsimd.tensor_scalar(TC[:], Q3[:], -0.25, 2.5, OP.mult, OP.add)

        # G = 0.5*Q1*TC as two Pool-legal ops (fills Pool slack; keeps DVE
        # free for the P2 tail chain).
        H = pool.tile([_P, _W], f32)
        nc.gpsimd.tensor_scalar_mul(H[:], Q1[:], 0.5)
        G = pool.tile([_P, _W], f32)
        nc.gpsimd.tensor_mul(G[:], H[:], TC[:])

        # P2: planes 0-2 accumulate on Pool right off SF012; planes 3-5
        # continue as a DVE stt chain straight into the D/R/FIN tail.
        WS = pool.tile([_P, 3 * _W], bf16)
        p2a_block = []
        for k in range(3):
            sl = slice(k * _W, (k + 1) * _W)
            p2a_block.append(nc.gpsimd.tensor_scalar_mul(WS[:, sl], SF[:, sl], L[k]))
        PA = pool.tile([_P, _W], bf16)
        p2a_block.append(nc.gpsimd.tensor_add(PA[:], WS[:, 0:_W], WS[:, _W : 2 * _W]))
        P2a = pool.tile([_P, _W], bf16)
        p2a_last = nc.gpsimd.tensor_add(P2a[:], PA[:], WS[:, 2 * _W : 3 * _W])
        p2a_block.append(p2a_last)
        # order-only edges: once SF012 lands, the P2a block takes priority
        # on Pool over the remaining Q3/SQ tail (whose consumers G/TC are
        # only needed ~700ns later)
        for late in q3_tail[-2:]:
            for early in p2a_block[-2:]:
                tile.add_dep_helper(
                    late.ins, early.ins, sync=False,
                    reason="P2a block before Q3 tail",
                )
        P2 = pool.tile([_P, _W], bf16)
        nc.vector.scalar_tensor_tensor(
            P2[:], SF[:, 3 * _W : 4 * _W], L[3], P2a[:], OP.mult, OP.add
        )
        nc.vector.scalar_tensor_tensor(
            P2[:], SF[:, 4 * _W : 5 * _W], L[4], P2[:], OP.mult, OP.add
        )
        nc.vector.scalar_tensor_tensor(
            P2[:], SF[:, 5 * _W : 6 * _W], L[5], P2[:], OP.mult, OP.add
        )

        # DVE tail: D = G - P2^2/32 (fused custom op); R ~= 1/D (1-pass
        # approx, ~51 ULP); cost = R*TC with the column sum fused via
        # accum_out.
        D = pool.tile([_P, _W], f32)
        nc.vector._custom_dve(d_op, out=D[:], in0=G[:], in1=P2[:], s0=1.0 / 32.0)
        R = pool.tile([_P, _W], f32)
        nc.vector.reciprocal_approx_fast(out=R[:], in_=D[:])
        COST = pool.tile([_P, _W], f32)
        CS = pool.tile([_P, 1], f32)
        nc.vector.scalar_tensor_tensor(
            COST[:], R[:], 1.0, TC[:], OP.mult, OP.mult, accum_out=CS[:]
        )
        nc.sync.dma_start(out_dram[:], CS[:])

    nc.compile()
    _CACHE["nc"] = nc
    return nc


def _shard(x):
    """Host prep: slice, cumsum over joints, /pi, RNE range-reduce to
    [-0.5,0.5], bf16, and lay out per core as [2 chunks, 128, 3*104]
    (chunk c = planes 3c..3c+2, k-major within the chunk)."""
    import ml_dtypes

    q = np.asarray(x[:, :_T, 6 : 6 + _K], dtype=np.float32)
    g = np.cumsum(q, axis=-1, dtype=np.float32) * np.float32(1.0 / np.pi)
    m = (g - np.rint(g)).astype(np.float32)
    # (B, T, K) -> (cores, K, P, W) point-major layout per plane
    planes = (
        m.reshape(_NCORES, _BPC * _T, _K)
        .transpose(0, 2, 1)
        .reshape(_NCORES, _K, _P, _W)
    )
    # group planes into chunks of 3: (cores, 2, 128, 312)
    chunks = (
        planes.reshape(_NCORES, _NCHUNK, 3, _P, _W)
        .transpose(0, 1, 3, 2, 4)
        .reshape(_NCORES, _NCHUNK, _P, _CW)
    )
    return np.ascontiguousarray(chunks.astype(ml_dtypes.bfloat16))


def _get_runner():
    """Build the jitted 8-core shard_map executable once (mirrors
    bass2jax.run_bass_via_pjrt's multi-core path) so repeat kernel() calls
    skip retracing/recompiling."""
    if "run" in _CACHE:
        return _CACHE["run"]
    import jax
    from jax.sharding import Mesh, PartitionSpec
    from jax.experimental.shard_map import shard_map
    from concourse import bass2jax

    nc = _get_nc()
    bass2jax.install_neuronx_cc_hook()
    assert nc.dbg_addr is None
    pid_name = nc.partition_id_tensor.name if nc.partition_id_tensor else None
    in_names = ("q", "out") + ((pid_name,) if pid_name else ())

    out_aval = jax.core.ShapedArray((_P, 1), np.float32)

    def _body(q, out_zero):
        operands = [q, out_zero]
        if pid_name is not None:
            operands.append(bass2jax.partition_id_tensor())
        (out,) = bass2jax._bass_exec_p.bind(
            *operands,
            out_avals=(out_aval,),
            in_names=in_names,
            out_names=("out",),
            lowering_input_output_aliases=(),
            sim_require_finite=True,
            sim_require_nnan=True,
            nc=nc,
        )
        return (out,)

    devices = jax.devices()[:_NCORES]
    mesh = Mesh(np.asarray(devices), ("core",))
    sharded = jax.jit(
        shard_map(
            _body,
            mesh=mesh,
            in_specs=(PartitionSpec("core"),) * 2,
            out_specs=(PartitionSpec("core"),),
            check_rep=False,
        ),
        donate_argnums=(1,),
        keep_unused=True,
    )

    def run(planes):
        concat_q = planes.reshape(_NCORES * _NCHUNK, _P, _CW)
        zeros = np.zeros((_NCORES * _P, 1), np.float32)
        (out,) = sharded(concat_q, zeros)
        return np.asarray(out)  # (8*128, 1)

    _CACHE["run"] = run
    return run


def _run_library(planes):
    from concourse.bass_utils import run_bass_kernel_spmd

    res = run_bass_kernel_spmd(
        _get_nc(),
        [{"q": planes[i]} for i in range(_NCORES)],
        list(range(_NCORES)),
    )
    return np.stack([r["out"][:, 0] for r in res.results]).astype(np.float32)


def _run_subprocess(planes):
    """Last resort: the accelerator occasionally reports
    NRT_EXEC_UNIT_UNRECOVERABLE; a fresh process reliably recovers it."""
    import os
    import subprocess
    import sys
    import tempfile

    d = tempfile.mkdtemp()
    inp = os.path.join(d, "planes.npy")
    out = os.path.join(d, "out.npy")
    np.save(inp, planes)
    here = os.path.dirname(os.path.abspath(__file__))
    script = (
        "import sys, numpy as np\n"
        f"sys.path.insert(0, {here!r})\n"
        "import kernel as K\n"
        f"planes = np.load({inp!r})\n"
        "out = K._get_runner()(planes)\n"
        f"np.save({out!r}, out)\n"
    )
    err = None
    for _ in range(2):
        try:
            subprocess.run(
                [sys.executable, "-c", script], check=True, timeout=900,
                stdout=subprocess.DEVNULL, stderr=subprocess.DEVNULL,
            )
            return np.load(out).astype(np.float32)
        except Exception as e:  # retry once; device usually recovers
            err = e
    raise err


def kernel(x, cond, time):
    x = np.asarray(x)
    planes = _shard(x)
    try:
        partials = _get_runner()(planes).astype(np.float32)
    except Exception:
        try:
            # library SPMD runner (covers fast-path/jax API drift)
            partials = _run_library(planes)
        except Exception:
            # fresh process recovers a wedged accelerator
            partials = _run_subprocess(planes)
    return np.float32(partials.sum(dtype=np.float32))
